# revision 1
# baseline (speedup 1.0000x reference)
"""Multi-head attention (B=2, T=2048, D=1024, H=16) on 8 TRN2 cores.

Sharding: core c -> batch b=c//4, head-group g=c%4 (4 heads, 256 proj cols).
Each core computes its 4 heads' attention + the partial out-projection
(O_g @ Wo[rows of g]); host sums the 4 partials per batch and adds
bo_eff = bo + bv @ Wo (exact fold: attention rows sum to 1, so bv passes
through attention unchanged; bk is softmax-invariant and dropped).

Device layout (per core):
  phase 1: project q,k,v streams (fp32r matmuls, contraction over D in
           8 blocks of 128) into Q^T/K^T [dh, T] tiles (2 tiles of 128
           partitions = 2 heads each) and V' tiles [128 t, 4*65] with a
           ones column per head (denominator trick).
  phase 2: per (tq-block j of 512, head hl): S^T = K^T_blk^T Q^T_blk,
           P = exp(S/8) * causal_mask, O'^T[65,512] += V'^T P
           (row 64 = sum of P = softmax denominator). Normalize via
           reciprocal + partition_broadcast.
  phase 3: out = O^T^T @ Wo_rows, accumulated over the 2 row-halves.
"""

import numpy as np
from contextlib import ExitStack

import concourse.tile as tile
from concourse import bacc, mybir
from concourse.bass_utils import run_bass_kernel_spmd

F32 = mybir.dt.float32
F32R = mybir.dt.float32r
AF = mybir.ActivationFunctionType

B, T, D, H, DH = 2, 2048, 1024, 16, 64
N_CORES = 8
HPC = 4            # heads per core
CS = HPC * DH      # 256 projection cols per core
NJ = T // 512      # 4 tq blocks
ND = D // 128      # 8 contraction blocks
NT = T // 128      # 16 t blocks
SCALE = 1.0 / 8.0  # 1/sqrt(DH)

_CACHE = {}


def _build():
    nc = bacc.Bacc("TRN2", target_bir_lowering=False, debug=False,
                   num_devices=N_CORES)
    qt_ap = nc.dram_tensor("qT", [D, T], F32, kind="ExternalInput").ap()
    kt_ap = nc.dram_tensor("kT", [D, T], F32, kind="ExternalInput").ap()
    vt_ap = nc.dram_tensor("vT", [D, T], F32, kind="ExternalInput").ap()
    wq_ap = nc.dram_tensor("wq", [D, CS], F32, kind="ExternalInput").ap()
    wk_ap = nc.dram_tensor("wk", [D, CS], F32, kind="ExternalInput").ap()
    wv_ap = nc.dram_tensor("wv", [D, CS], F32, kind="ExternalInput").ap()
    wo_ap = nc.dram_tensor("wo", [CS, D], F32, kind="ExternalInput").ap()
    bq_ap = nc.dram_tensor("bq", [CS], F32, kind="ExternalInput").ap()
    out_ap = nc.dram_tensor("out", [T, D], F32, kind="ExternalOutput").ap()

    with tile.TileContext(nc) as tc, ExitStack() as ctx, \
            nc.allow_low_precision(reason="fp32r attention pipeline"):
        per = ctx.enter_context(tc.tile_pool(name="per", bufs=1))
        stream = ctx.enter_context(tc.tile_pool(name="stream", bufs=1))
        work = ctx.enter_context(tc.tile_pool(name="work", bufs=1))
        ps_pj = ctx.enter_context(tc.tile_pool(name="ps_pj", bufs=2, space="PSUM"))
        ps_s = ctx.enter_context(tc.tile_pool(name="ps_s", bufs=2, space="PSUM"))
        ps_o = ctx.enter_context(tc.tile_pool(name="ps_o", bufs=2, space="PSUM"))
        ps_op = ctx.enter_context(tc.tile_pool(name="ps_op", bufs=2, space="PSUM"))

        # ---- weights / constants ----
        wq_sb = per.tile([128, ND * CS], F32R)
        wk_sb = per.tile([128, ND * CS], F32R)
        wv_sb = per.tile([128, ND * CS], F32R)
        for i in range(ND):
            nc.gpsimd.dma_start(wq_sb[:, CS * i:CS * (i + 1)], wq_ap[128 * i:128 * (i + 1), :])
            nc.gpsimd.dma_start(wk_sb[:, CS * i:CS * (i + 1)], wk_ap[128 * i:128 * (i + 1), :])
            nc.gpsimd.dma_start(wv_sb[:, CS * i:CS * (i + 1)], wv_ap[128 * i:128 * (i + 1), :])
        wo_sb = []
        for ct in range(2):
            t = per.tile([128, D], F32R, name=f"wo{ct}")
            nc.gpsimd.dma_start(t[:], wo_ap[128 * ct:128 * (ct + 1), :])
            wo_sb.append(t)
        bq_sb = []
        for ct in range(2):
            t = per.tile([128, 1], F32, name=f"bq{ct}")
            nc.gpsimd.dma_start(t[:], bq_ap[128 * ct:128 * (ct + 1)].unsqueeze(1))
            bq_sb.append(t)
        ones_sb = per.tile([128, HPC], F32)
        nc.gpsimd.memset(ones_sb[:], 1.0)
        masks = []
        for r in range(4):
            m = per.tile([128, 512], F32, name=f"mask{r}")
            nc.gpsimd.memset(m[:], 1.0)
            nc.gpsimd.affine_select(
                out=m[:], in_=m[:], compare_op=mybir.AluOpType.is_ge,
                fill=0.0, base=-128 * r, pattern=[[1, 512]], channel_multiplier=-1,
            )
            masks.append(m)

        qT_sb = [per.tile([128, T], F32R, name=f"qT{ct}") for ct in range(2)]
        kT_sb = [per.tile([128, T], F32R, name=f"kT{ct}") for ct in range(2)]
        oT_sb = [per.tile([128, T], F32R, name=f"oT{ct}") for ct in range(2)]
        vp_sb = [per.tile([128, HPC * 65], F32R, name=f"vp{tt}") for tt in range(NT)]
        for tt in range(NT):
            nc.scalar.activation(
                vp_sb[tt].rearrange("p (h x) -> p h x", h=HPC)[:, :, 64:65],
                ones_sb.rearrange("p (h x) -> p h x", x=1),
                AF.Copy,
            )

        # ---- phase 1: projections ----
        for j in range(NJ):
            qs, ks, vs = [], [], []
            for i in range(ND):
                tq_ = stream.tile([128, 512], F32R, name=f"qs{i}")
                nc.gpsimd.dma_start(tq_[:], qt_ap[128 * i:128 * (i + 1), 512 * j:512 * (j + 1)])
                qs.append(tq_)
                tk_ = stream.tile([128, 512], F32R, name=f"ks{i}")
                nc.gpsimd.dma_start(tk_[:], kt_ap[128 * i:128 * (i + 1), 512 * j:512 * (j + 1)])
                ks.append(tk_)
                tv_ = stream.tile([128, 512], F32R, name=f"vs{i}")
                nc.gpsimd.dma_start(tv_[:], vt_ap[128 * i:128 * (i + 1), 512 * j:512 * (j + 1)])
                vs.append(tv_)

            for w_sb, src, dst, bias in (
                (wq_sb, qs, qT_sb, bq_sb),
                (wk_sb, ks, kT_sb, None),
            ):
                for ct in range(2):
                    ps = ps_pj.tile([128, 512], F32, name="pj_ps")
                    for i in range(ND):
                        nc.tensor.matmul(
                            ps[:],
                            w_sb[:, CS * i + 128 * ct:CS * i + 128 * ct + 128],
                            src[i][:],
                            start=(i == 0), stop=(i == ND - 1),
                        )
                    if bias is not None:
                        nc.scalar.activation(
                            dst[ct][:, 512 * j:512 * (j + 1)], ps[:],
                            AF.Identity, bias=bias[ct][:, 0:1], scale=1.0,
                        )
                    else:
                        nc.scalar.activation(
                            dst[ct][:, 512 * j:512 * (j + 1)], ps[:], AF.Copy,
                        )

            for u in range(4):
                tt = 4 * j + u
                ps = ps_pj.tile([128, 512], F32, name="pj_ps")
                for i in range(ND):
                    nc.tensor.matmul(
                        ps[:, 0:CS],
                        vs[i][:, 128 * u:128 * (u + 1)],
                        wv_sb[:, CS * i:CS * (i + 1)],
                        start=(i == 0), stop=(i == ND - 1),
                    )
                nc.vector.tensor_copy(
                    vp_sb[tt].rearrange("p (h x) -> p h x", h=HPC)[:, :, 0:64],
                    ps[:, 0:CS].rearrange("p (h x) -> p h x", h=HPC),
                )

        # ---- phase 2: attention ----
        for j in range(NJ):
            for hl in range(HPC):
                ct, po = hl // 2, 64 * (hl % 2)
                n_i = 4 * j + 4
                o_ps = ps_o.tile([65, 512], F32, name="o_ps")
                for i in range(n_i):
                    s_ps = ps_s.tile([128, 512], F32, name="s_ps")
                    nc.tensor.matmul(
                        s_ps[:],
                        kT_sb[ct][po:po + 64, 128 * i:128 * (i + 1)],
                        qT_sb[ct][po:po + 64, 512 * j:512 * (j + 1)],
                        start=True, stop=True,
                    )
                    p_sb = work.tile([128, 512], F32R, name="p_sb", bufs=4)
                    nc.scalar.activation(p_sb[:], s_ps[:], AF.Exp, scale=SCALE)
                    if i >= 4 * j:
                        nc.vector.tensor_mul(p_sb[:], p_sb[:], masks[i - 4 * j][:])
                    nc.tensor.matmul(
                        o_ps[:], vp_sb[i][:, 65 * hl:65 * hl + 65], p_sb[:],
                        start=(i == 0), stop=(i == n_i - 1), skip_group_check=True,
                    )
                recip = work.tile([128, 512], F32R, name="recip", bufs=2)
                nc.vector.reciprocal(recip[64:65, :], o_ps[64:65, :])
                r0 = work.tile([128, 512], F32R, name="r0", bufs=2)
                nc.gpsimd.dma_start(r0[0:1, :], recip[64:65, :])
                bcast = work.tile([128, 512], F32R, name="bcast", bufs=2)
                nc.gpsimd.partition_broadcast(bcast[0:64, :], r0[0:1, :])
                if po == 0:
                    nc.vector.tensor_mul(
                        oT_sb[ct][0:64, 512 * j:512 * (j + 1)],
                        o_ps[0:64, :], bcast[0:64, :],
                    )
                else:
                    stage = work.tile([128, 512], F32R, name="stage", bufs=2)
                    nc.vector.tensor_mul(stage[0:64, :], o_ps[0:64, :], bcast[0:64, :])
                    nc.gpsimd.dma_start(
                        oT_sb[ct][64:128, 512 * j:512 * (j + 1)], stage[0:64, :],
                    )

        # ---- phase 3: out-projection partial ----
        for tt in range(NT):
            for do in range(2):
                op_ps = ps_op.tile([128, 512], F32, name="op_ps")
                for ct in range(2):
                    nc.tensor.matmul(
                        op_ps[:],
                        oT_sb[ct][:, 128 * tt:128 * (tt + 1)],
                        wo_sb[ct][:, 512 * do:512 * (do + 1)],
                        start=(ct == 0), stop=(ct == 1),
                    )
                od = work.tile([128, 512], F32, name="odrain", bufs=2)
                nc.vector.tensor_copy(od[:], op_ps[:])
                nc.gpsimd.dma_start(
                    out_ap[128 * tt:128 * (tt + 1), 512 * do:512 * (do + 1)], od[:],
                )

    nc.compile()
    return nc


def _get_nc():
    if "nc" not in _CACHE:
        _CACHE["nc"] = _build()
    return _CACHE["nc"]


def kernel(**inputs):
    q = np.asarray(inputs["q"], np.float32)
    k = np.asarray(inputs["k"], np.float32)
    v = np.asarray(inputs["v"], np.float32)
    Wq = np.asarray(inputs["Wq"], np.float32)
    Wk = np.asarray(inputs["Wk"], np.float32)
    Wv = np.asarray(inputs["Wv"], np.float32)
    Wo = np.asarray(inputs["Wo"], np.float32)
    bq = np.asarray(inputs["bq"], np.float32)
    bv = np.asarray(inputs["bv"], np.float32)
    bo = np.asarray(inputs["bo"], np.float32)

    nc = _get_nc()
    qT = [np.ascontiguousarray(q[b].T) for b in range(B)]
    kT = [np.ascontiguousarray(k[b].T) for b in range(B)]
    vT = [np.ascontiguousarray(v[b].T) for b in range(B)]
    in_maps = []
    for c in range(N_CORES):
        b, g = c // 4, c % 4
        cs = CS * g
        in_maps.append({
            "qT": qT[b], "kT": kT[b], "vT": vT[b],
            "wq": np.ascontiguousarray(Wq[:, cs:cs + CS]),
            "wk": np.ascontiguousarray(Wk[:, cs:cs + CS]),
            "wv": np.ascontiguousarray(Wv[:, cs:cs + CS]),
            "wo": np.ascontiguousarray(Wo[cs:cs + CS, :]),
            "bq": np.ascontiguousarray(bq[cs:cs + CS]),
        })
    res = run_bass_kernel_spmd(nc, in_maps, list(range(N_CORES)))

    bo_eff = bo.astype(np.float64) + bv.astype(np.float64) @ Wo.astype(np.float64)
    out = np.empty((B, T, D), np.float32)
    for b in range(B):
        acc = np.zeros((T, D), np.float64)
        for g in range(HPC):
            acc += res.results[4 * b + g]["out"].astype(np.float64)
        out[b] = (acc + bo_eff).astype(np.float32)
    return out


# revision 4
# speedup vs baseline: 1.4067x; 1.4067x over previous
"""Multi-head attention (B=2, T=2048, D=1024, H=16) on 8 TRN2 cores.

Sharding: core c -> batch b=c//4, head-group g=c%4 (4 heads, 256 proj cols).
Each core computes its 4 heads' attention + the partial out-projection
(O_g @ Wo[rows of g]); host sums the 4 partials per batch and adds
bo_eff = bo + bv @ Wo (exact fold: attention rows sum to 1, so bv passes
through attention unchanged; bk is softmax-invariant and dropped).

Device pipeline, per tq-block j of 512 (phases interleaved so attention of
block j overlaps the q/k/v streaming of block j+1):
  proj:  Q^T/K^T [dh, T] fp32r tiles (2 x 128 partitions = 2 heads each) and
         V' tiles [128 t, 4*65] with a ones column per head (denominator
         trick: row 64 of the PV accumulator is the softmax denominator).
  attn:  per head hl: S^T = K^T_blk^T Q^T_blk, P = exp(S/8) * causal_mask,
         O'^T[65,512] += V'^T P. Normalize via reciprocal of row 64 +
         partition_broadcast (input must sit at partition 0, hence the DMA
         hop).
  oproj: out[128t, 1024] = O^T^T @ Wo_rows accumulated over both row-halves.

All DMAs are issued from the SP engine (HWDGE): SWDGE on Pool costs ~1us
fixed per DMA and was the original bottleneck.
"""

import numpy as np
from contextlib import ExitStack

import concourse.tile as tile
from concourse import bacc, mybir
from concourse.bass_utils import run_bass_kernel_spmd

F32 = mybir.dt.float32
F32R = mybir.dt.float32r
AF = mybir.ActivationFunctionType

B, T, D, H, DH = 2, 2048, 1024, 16, 64
N_CORES = 8
HPC = 4            # heads per core
CS = HPC * DH      # 256 projection cols per core
NJ = T // 512      # 4 tq blocks
ND = D // 128      # 8 contraction blocks
NT = T // 128      # 16 t blocks
SCALE = 1.0 / 8.0  # 1/sqrt(DH)

_CACHE = {}


def _build():
    nc = bacc.Bacc("TRN2", target_bir_lowering=False, debug=False,
                   num_devices=N_CORES)
    qt_ap = nc.dram_tensor("qT", [D, T], F32, kind="ExternalInput").ap()
    kt_ap = nc.dram_tensor("kT", [D, T], F32, kind="ExternalInput").ap()
    vt_ap = nc.dram_tensor("vT", [D, T], F32, kind="ExternalInput").ap()
    wq_ap = nc.dram_tensor("wq", [D, CS], F32, kind="ExternalInput").ap()
    wk_ap = nc.dram_tensor("wk", [D, CS], F32, kind="ExternalInput").ap()
    wv_ap = nc.dram_tensor("wv", [D, CS], F32, kind="ExternalInput").ap()
    wo_ap = nc.dram_tensor("wo", [CS, D], F32, kind="ExternalInput").ap()
    bq_ap = nc.dram_tensor("bq", [CS], F32, kind="ExternalInput").ap()
    out_ap = nc.dram_tensor("out", [T, D], F32, kind="ExternalOutput").ap()

    with tile.TileContext(nc) as tc, ExitStack() as ctx, \
            nc.allow_low_precision(reason="fp32r attention pipeline"):
        per = ctx.enter_context(tc.tile_pool(name="per", bufs=1))
        stream = ctx.enter_context(tc.tile_pool(name="stream", bufs=1))
        work = ctx.enter_context(tc.tile_pool(name="work", bufs=1))
        ps_pj = ctx.enter_context(tc.tile_pool(name="ps_pj", bufs=2, space="PSUM"))
        ps_s = ctx.enter_context(tc.tile_pool(name="ps_s", bufs=2, space="PSUM"))
        ps_o = ctx.enter_context(tc.tile_pool(name="ps_o", bufs=2, space="PSUM"))
        ps_op = ctx.enter_context(tc.tile_pool(name="ps_op", bufs=2, space="PSUM"))

        # ---- weights / constants ----
        wq_sb = per.tile([128, ND * CS], F32R)
        wk_sb = per.tile([128, ND * CS], F32R)
        wv_sb = per.tile([128, ND * CS], F32R)
        for ap_, sb_ in ((wq_ap, wq_sb), (wk_ap, wk_sb), (wv_ap, wv_sb)):
            nc.sync.dma_start(
                sb_.rearrange("p (i c) -> p i c", i=ND),
                ap_.rearrange("(i p) c -> p i c", p=128).bitcast(F32R),
            )
        wo_sb = []
        for ct in range(2):
            t = per.tile([128, D], F32R, name=f"wo{ct}")
            nc.sync.dma_start(t[:], wo_ap[128 * ct:128 * (ct + 1), :].bitcast(F32R))
            wo_sb.append(t)
        bq_sb = []
        for ct in range(2):
            t = per.tile([128, 1], F32, name=f"bq{ct}")
            nc.sync.dma_start(t[:], bq_ap[128 * ct:128 * (ct + 1)].unsqueeze(1))
            bq_sb.append(t)
        ones_sb = per.tile([128, HPC], F32)
        nc.gpsimd.memset(ones_sb[:], 1.0)
        masks = []
        for r in range(4):
            m = per.tile([128, 512], F32, name=f"mask{r}")
            nc.gpsimd.memset(m[:], 1.0)
            nc.gpsimd.affine_select(
                out=m[:], in_=m[:], compare_op=mybir.AluOpType.is_ge,
                fill=0.0, base=-128 * r, pattern=[[1, 512]], channel_multiplier=-1,
            )
            masks.append(m)

        qT_sb = [per.tile([128, T], F32R, name=f"qT{ct}") for ct in range(2)]
        kT_sb = [per.tile([128, T], F32R, name=f"kT{ct}") for ct in range(2)]
        oT_sb = [per.tile([128, T], F32R, name=f"oT{ct}") for ct in range(2)]
        vp_sb = [per.tile([128, HPC * 65], F32R, name=f"vp{tt}") for tt in range(NT)]
        for tt in range(NT):
            nc.scalar.activation(
                vp_sb[tt].rearrange("p (h x) -> p h x", h=HPC)[:, :, 64:65],
                ones_sb.rearrange("p (h x) -> p h x", x=1),
                AF.Copy,
            )

        for j in range(NJ):
            # ---- stream in q/k/v columns for this t-block (one DMA each) ----
            qs = stream.tile([128, ND * 512], F32R, name="qs")
            ks = stream.tile([128, ND * 512], F32R, name="ks")
            vs = stream.tile([128, ND * 512], F32R, name="vs")
            for ap_, sb_ in ((qt_ap, qs), (kt_ap, ks), (vt_ap, vs)):
                nc.sync.dma_start(
                    sb_.rearrange("p (i t) -> p i t", i=ND),
                    ap_.rearrange("(i p) t -> p i t", p=128)
                       [:, :, 512 * j:512 * (j + 1)].bitcast(F32R),
                )

            # ---- Q/K projections ----
            for w_sb, src, dst, bias in (
                (wq_sb, qs, qT_sb, bq_sb),
                (wk_sb, ks, kT_sb, None),
            ):
                for ct in range(2):
                    ps = ps_pj.tile([128, 512], F32, name="pj_ps")
                    for i in range(ND):
                        nc.tensor.matmul(
                            ps[:],
                            w_sb[:, CS * i + 128 * ct:CS * i + 128 * ct + 128],
                            src[:, 512 * i:512 * (i + 1)],
                            start=(i == 0), stop=(i == ND - 1),
                        )
                    if bias is not None:
                        nc.scalar.activation(
                            dst[ct][:, 512 * j:512 * (j + 1)], ps[:],
                            AF.Identity, bias=bias[ct][:, 0:1], scale=1.0,
                        )
                    else:
                        nc.scalar.activation(
                            dst[ct][:, 512 * j:512 * (j + 1)], ps[:], AF.Copy,
                        )

            # ---- V projection ----
            for u in range(4):
                tt = 4 * j + u
                ps = ps_pj.tile([128, 512], F32, name="pj_ps")
                for i in range(ND):
                    nc.tensor.matmul(
                        ps[:, 0:CS],
                        vs[:, 512 * i + 128 * u:512 * i + 128 * (u + 1)],
                        wv_sb[:, CS * i:CS * (i + 1)],
                        start=(i == 0), stop=(i == ND - 1),
                    )
                nc.vector.tensor_copy(
                    vp_sb[tt].rearrange("p (h x) -> p h x", h=HPC)[:, :, 0:64],
                    ps[:, 0:CS].rearrange("p (h x) -> p h x", h=HPC),
                )

            # ---- attention for tq block j ----
            for hl in range(HPC):
                ct, po = hl // 2, 64 * (hl % 2)
                n_i = 4 * j + 4
                o_ps = ps_o.tile([65, 512], F32, name="o_ps")
                for i in range(n_i):
                    s_ps = ps_s.tile([128, 512], F32, name="s_ps")
                    nc.tensor.matmul(
                        s_ps[:],
                        kT_sb[ct][po:po + 64, 128 * i:128 * (i + 1)],
                        qT_sb[ct][po:po + 64, 512 * j:512 * (j + 1)],
                        start=True, stop=True,
                    )
                    p_sb = work.tile([128, 512], F32R, name="p_sb", bufs=4)
                    nc.scalar.activation(p_sb[:], s_ps[:], AF.Exp, scale=SCALE)
                    if i >= 4 * j:
                        nc.vector.tensor_mul(p_sb[:], p_sb[:], masks[i - 4 * j][:])
                    nc.tensor.matmul(
                        o_ps[:], vp_sb[i][:, 65 * hl:65 * hl + 65], p_sb[:],
                        start=(i == 0), stop=(i == n_i - 1), skip_group_check=True,
                    )
                recip = work.tile([128, 512], F32R, name="recip", bufs=2)
                nc.vector.reciprocal(recip[64:65, :], o_ps[64:65, :])
                r0 = work.tile([128, 512], F32R, name="r0", bufs=2)
                nc.sync.dma_start(r0[0:1, :], recip[64:65, :])
                bcast = work.tile([128, 512], F32R, name="bcast", bufs=2)
                nc.gpsimd.partition_broadcast(bcast[0:64, :], r0[0:1, :])
                if po == 0:
                    nc.vector.tensor_mul(
                        oT_sb[ct][0:64, 512 * j:512 * (j + 1)],
                        o_ps[0:64, :], bcast[0:64, :],
                    )
                else:
                    stage = work.tile([128, 512], F32R, name="stage", bufs=2)
                    nc.vector.tensor_mul(stage[0:64, :], o_ps[0:64, :], bcast[0:64, :])
                    nc.sync.dma_start(
                        oT_sb[ct][64:128, 512 * j:512 * (j + 1)], stage[0:64, :],
                    )

            # ---- out-projection partial for this tq block ----
            for u in range(4):
                tt = 4 * j + u
                od = work.tile([128, D], F32, name="odrain", bufs=2)
                for do in range(2):
                    op_ps = ps_op.tile([128, 512], F32, name="op_ps")
                    for ct in range(2):
                        nc.tensor.matmul(
                            op_ps[:],
                            oT_sb[ct][:, 128 * tt:128 * (tt + 1)],
                            wo_sb[ct][:, 512 * do:512 * (do + 1)],
                            start=(ct == 0), stop=(ct == 1),
                        )
                    nc.vector.tensor_copy(od[:, 512 * do:512 * (do + 1)], op_ps[:])
                nc.sync.dma_start(out_ap[128 * tt:128 * (tt + 1), :], od[:])

    nc.compile()
    return nc


def _get_nc():
    if "nc" not in _CACHE:
        _CACHE["nc"] = _build()
    return _CACHE["nc"]


def kernel(**inputs):
    q = np.asarray(inputs["q"], np.float32)
    k = np.asarray(inputs["k"], np.float32)
    v = np.asarray(inputs["v"], np.float32)
    Wq = np.asarray(inputs["Wq"], np.float32)
    Wk = np.asarray(inputs["Wk"], np.float32)
    Wv = np.asarray(inputs["Wv"], np.float32)
    Wo = np.asarray(inputs["Wo"], np.float32)
    bq = np.asarray(inputs["bq"], np.float32)
    bv = np.asarray(inputs["bv"], np.float32)
    bo = np.asarray(inputs["bo"], np.float32)

    nc = _get_nc()
    qT = [np.ascontiguousarray(q[b].T) for b in range(B)]
    kT = [np.ascontiguousarray(k[b].T) for b in range(B)]
    vT = [np.ascontiguousarray(v[b].T) for b in range(B)]
    in_maps = []
    for c in range(N_CORES):
        b, g = c // 4, c % 4
        cs = CS * g
        in_maps.append({
            "qT": qT[b], "kT": kT[b], "vT": vT[b],
            "wq": np.ascontiguousarray(Wq[:, cs:cs + CS]),
            "wk": np.ascontiguousarray(Wk[:, cs:cs + CS]),
            "wv": np.ascontiguousarray(Wv[:, cs:cs + CS]),
            "wo": np.ascontiguousarray(Wo[cs:cs + CS, :]),
            "bq": np.ascontiguousarray(bq[cs:cs + CS]),
        })
    res = run_bass_kernel_spmd(nc, in_maps, list(range(N_CORES)))

    bo_eff = bo.astype(np.float64) + bv.astype(np.float64) @ Wo.astype(np.float64)
    out = np.empty((B, T, D), np.float32)
    for b in range(B):
        acc = np.zeros((T, D), np.float64)
        for g in range(HPC):
            acc += res.results[4 * b + g]["out"].astype(np.float64)
        out[b] = (acc + bo_eff).astype(np.float32)
    return out


# revision 10
# speedup vs baseline: 1.4241x; 1.0124x over previous
"""Multi-head attention (B=2, T=2048, D=1024, H=16) on 8 TRN2 cores.

Sharding: core c -> batch b=c//4, head-group g=c%4 (4 heads, 256 proj cols).
Each core computes its 4 heads' attention + the partial out-projection
(O_g @ Wo[rows of g]); host sums the 4 partials per batch and adds
bo_eff = bo + bv @ Wo (exact fold: attention rows sum to 1, so bv passes
through attention unchanged; bk is softmax-invariant and dropped).

Device pipeline, per tq-block j of 512 (phases interleaved so attention of
block j overlaps the q/k/v streaming of block j+1):
  proj:  Q^T/K^T [dh, T] fp32r tiles (2 x 128 partitions = 2 heads each) and
         V' tiles [128 t, 4*65] with a ones column per head (denominator
         trick: row 64 of the PV accumulator is the softmax denominator).
  attn:  per head hl: S^T = K^T_blk^T Q^T_blk, P = exp(S/8) * causal_mask,
         O'^T[65,512] += V'^T P. Normalize via reciprocal of row 64 +
         partition_broadcast (input must sit at partition 0, hence the DMA
         hop).
  oproj: out[128t, 1024] = O^T^T @ Wo_rows accumulated over both row-halves.

All DMAs are issued from the SP engine (HWDGE): SWDGE on Pool costs ~1us
fixed per DMA and was the original bottleneck.
"""

import numpy as np
from contextlib import ExitStack

import concourse.tile as tile
from concourse import bacc, mybir
from concourse.bass_utils import run_bass_kernel_spmd

F32 = mybir.dt.float32
F32R = mybir.dt.float32r
AF = mybir.ActivationFunctionType

B, T, D, H, DH = 2, 2048, 1024, 16, 64
N_CORES = 8
HPC = 4            # heads per core
CS = HPC * DH      # 256 projection cols per core
NJ = T // 512      # 4 tq blocks
ND = D // 128      # 8 contraction blocks
NT = T // 128      # 16 t blocks
SCALE = 1.0 / 8.0  # 1/sqrt(DH)

_CACHE = {}


def _build():
    nc = bacc.Bacc("TRN2", target_bir_lowering=False, debug=False,
                   num_devices=N_CORES)
    qt_ap = nc.dram_tensor("qT", [D, T], F32, kind="ExternalInput").ap()
    kt_ap = nc.dram_tensor("kT", [D, T], F32, kind="ExternalInput").ap()
    vt_ap = nc.dram_tensor("vT", [D, T], F32, kind="ExternalInput").ap()
    wq_ap = nc.dram_tensor("wq", [D, CS], F32, kind="ExternalInput").ap()
    wk_ap = nc.dram_tensor("wk", [D, CS], F32, kind="ExternalInput").ap()
    wv_ap = nc.dram_tensor("wv", [D, CS], F32, kind="ExternalInput").ap()
    wo_ap = nc.dram_tensor("wo", [CS, D], F32, kind="ExternalInput").ap()
    bq_ap = nc.dram_tensor("bq", [CS], F32, kind="ExternalInput").ap()
    out_ap = nc.dram_tensor("out", [T, D], F32, kind="ExternalOutput").ap()

    with tile.TileContext(nc) as tc, ExitStack() as ctx, \
            nc.allow_low_precision(reason="fp32r attention pipeline"):
        per = ctx.enter_context(tc.tile_pool(name="per", bufs=1))
        stream = ctx.enter_context(tc.tile_pool(name="stream", bufs=1))
        work = ctx.enter_context(tc.tile_pool(name="work", bufs=1))
        ps_pj = ctx.enter_context(tc.tile_pool(name="ps_pj", bufs=2, space="PSUM"))
        ps_s = ctx.enter_context(tc.tile_pool(name="ps_s", bufs=2, space="PSUM"))
        ps_o = ctx.enter_context(tc.tile_pool(name="ps_o", bufs=2, space="PSUM"))
        ps_op = ctx.enter_context(tc.tile_pool(name="ps_op", bufs=2, space="PSUM"))

        # ---- weight / stream loads, interleaved in dependency order ----
        def load_stream(jj):
            qs = stream.tile([128, ND * 512], F32R, name="qs")
            ks = stream.tile([128, ND * 512], F32R, name="ks")
            vs = stream.tile([128, ND * 512], F32R, name="vs")
            for ap_, sb_ in ((qt_ap, qs), (kt_ap, ks), (vt_ap, vs)):
                nc.sync.dma_start(
                    sb_.rearrange("p (i t) -> p i t", i=ND),
                    ap_.rearrange("(i p) t -> p i t", p=128)
                       [:, :, 512 * jj:512 * (jj + 1)].bitcast(F32R),
                )
            return qs, ks, vs

        wq_sb = per.tile([128, ND * CS], F32R)
        wk_sb = per.tile([128, ND * CS], F32R)
        wv_sb = per.tile([128, ND * CS], F32R)
        for ap_, sb_ in ((wq_ap, wq_sb), (wk_ap, wk_sb), (wv_ap, wv_sb)):
            nc.sync.dma_start(
                sb_.rearrange("p (i c) -> p i c", i=ND),
                ap_.rearrange("(i p) c -> p i c", p=128).bitcast(F32R),
            )
        cur = load_stream(0)
        wo_sb = []
        for ct in range(2):
            t = per.tile([128, D], F32R, name=f"wo{ct}")
            nc.sync.dma_start(t[:], wo_ap[128 * ct:128 * (ct + 1), :].bitcast(F32R))
            wo_sb.append(t)
        bq_sb = []
        for ct in range(2):
            t = per.tile([128, 1], F32, name=f"bq{ct}")
            nc.sync.dma_start(t[:], bq_ap[128 * ct:128 * (ct + 1)].unsqueeze(1))
            bq_sb.append(t)
        ones_sb = per.tile([128, HPC], F32)
        nc.gpsimd.memset(ones_sb[:], 1.0)
        masks = []
        for r in range(4):
            m = per.tile([128, 512], F32, name=f"mask{r}")
            nc.gpsimd.memset(m[:], 1.0)
            nc.gpsimd.affine_select(
                out=m[:], in_=m[:], compare_op=mybir.AluOpType.is_ge,
                fill=0.0, base=-128 * r, pattern=[[1, 512]], channel_multiplier=-1,
            )
            masks.append(m)

        qT_sb = [per.tile([128, T], F32R, name=f"qT{ct}") for ct in range(2)]
        kT_sb = [per.tile([128, T], F32R, name=f"kT{ct}") for ct in range(2)]
        oT_sb = [per.tile([128, T], F32R, name=f"oT{ct}") for ct in range(2)]
        vp_sb = [per.tile([128, HPC * 65], F32R, name=f"vp{tt}") for tt in range(NT)]
        for tt in range(NT):
            nc.scalar.activation(
                vp_sb[tt].rearrange("p (h x) -> p h x", h=HPC)[:, :, 64:65],
                ones_sb.rearrange("p (h x) -> p h x", x=1),
                AF.Copy,
            )

        for j in range(NJ):
            qs, ks, vs = cur

            # ---- Q/K projections ----
            for w_sb, src, dst, bias in (
                (wq_sb, qs, qT_sb, bq_sb),
                (wk_sb, ks, kT_sb, None),
            ):
                for ct in range(2):
                    ps = ps_pj.tile([128, 512], F32, name="pj_ps")
                    for i in range(ND):
                        nc.tensor.matmul(
                            ps[:],
                            w_sb[:, CS * i + 128 * ct:CS * i + 128 * ct + 128],
                            src[:, 512 * i:512 * (i + 1)],
                            start=(i == 0), stop=(i == ND - 1),
                        )
                    if bias is not None:
                        nc.vector.tensor_scalar_add(
                            dst[ct][:, 512 * j:512 * (j + 1)],
                            ps[:], bias[ct][:, 0:1],
                        )
                    else:
                        nc.vector.tensor_copy(
                            dst[ct][:, 512 * j:512 * (j + 1)], ps[:],
                        )

            # ---- V projection ----
            for u in range(4):
                tt = 4 * j + u
                ps = ps_pj.tile([128, 512], F32, name="pj_ps")
                for i in range(ND):
                    nc.tensor.matmul(
                        ps[:, 0:CS],
                        vs[:, 512 * i + 128 * u:512 * i + 128 * (u + 1)],
                        wv_sb[:, CS * i:CS * (i + 1)],
                        start=(i == 0), stop=(i == ND - 1),
                    )
                nc.vector.tensor_copy(
                    vp_sb[tt].rearrange("p (h x) -> p h x", h=HPC)[:, :, 0:64],
                    ps[:, 0:CS].rearrange("p (h x) -> p h x", h=HPC),
                )

            # prefetch next block's q/k/v now that this block's are consumed
            if j + 1 < NJ:
                cur = load_stream(j + 1)

            # ---- attention for tq block j ----
            for hl in range(HPC):
                ct, po = hl // 2, 64 * (hl % 2)
                n_i = 4 * j + 4
                o_ps = ps_o.tile([65, 512], F32, name="o_ps")
                for i in range(n_i):
                    s_ps = ps_s.tile([128, 512], F32, name="s_ps")
                    nc.tensor.matmul(
                        s_ps[:],
                        kT_sb[ct][po:po + 64, 128 * i:128 * (i + 1)],
                        qT_sb[ct][po:po + 64, 512 * j:512 * (j + 1)],
                        start=True, stop=True,
                    )
                    p_sb = work.tile([128, 512], F32R, name="p_sb", bufs=4)
                    nc.scalar.activation(p_sb[:], s_ps[:], AF.Exp, scale=SCALE)
                    if i >= 4 * j:
                        nc.vector.tensor_mul(p_sb[:], p_sb[:], masks[i - 4 * j][:])
                    nc.tensor.matmul(
                        o_ps[:], vp_sb[i][:, 65 * hl:65 * hl + 65], p_sb[:],
                        start=(i == 0), stop=(i == n_i - 1), skip_group_check=True,
                    )
                recip = work.tile([128, 512], F32, name="recip", bufs=2)
                nc.vector.reciprocal(recip[64:65, :], o_ps[64:65, :])
                r0 = work.tile([128, 512], F32, name="r0", bufs=2)
                nc.sync.dma_start(r0[0:1, :], recip[64:65, :])
                bcast = work.tile([128, 512], F32, name="bcast", bufs=2)
                nc.gpsimd.partition_broadcast(bcast[0:64, :], r0[0:1, :])
                if po == 0:
                    nc.vector.tensor_mul(
                        oT_sb[ct][0:64, 512 * j:512 * (j + 1)],
                        o_ps[0:64, :], bcast[0:64, :],
                    )
                else:
                    stage = work.tile([128, 512], F32R, name="stage", bufs=2)
                    nc.vector.tensor_mul(stage[0:64, :], o_ps[0:64, :], bcast[0:64, :])
                    nc.sync.dma_start(
                        oT_sb[ct][64:128, 512 * j:512 * (j + 1)], stage[0:64, :],
                    )

            # ---- out-projection partial for this tq block ----
            for u in range(4):
                tt = 4 * j + u
                od = work.tile([128, D], F32, name="odrain", bufs=2)
                for do in range(2):
                    op_ps = ps_op.tile([128, 512], F32, name="op_ps")
                    for ct in range(2):
                        nc.tensor.matmul(
                            op_ps[:],
                            oT_sb[ct][:, 128 * tt:128 * (tt + 1)],
                            wo_sb[ct][:, 512 * do:512 * (do + 1)],
                            start=(ct == 0), stop=(ct == 1),
                        )
                    nc.vector.tensor_copy(od[:, 512 * do:512 * (do + 1)], op_ps[:])
                nc.sync.dma_start(out_ap[128 * tt:128 * (tt + 1), :], od[:])

    nc.compile()
    return nc


def _get_nc():
    if "nc" not in _CACHE:
        _CACHE["nc"] = _build()
    return _CACHE["nc"]


def kernel(**inputs):
    q = np.asarray(inputs["q"], np.float32)
    k = np.asarray(inputs["k"], np.float32)
    v = np.asarray(inputs["v"], np.float32)
    Wq = np.asarray(inputs["Wq"], np.float32)
    Wk = np.asarray(inputs["Wk"], np.float32)
    Wv = np.asarray(inputs["Wv"], np.float32)
    Wo = np.asarray(inputs["Wo"], np.float32)
    bq = np.asarray(inputs["bq"], np.float32)
    bv = np.asarray(inputs["bv"], np.float32)
    bo = np.asarray(inputs["bo"], np.float32)

    nc = _get_nc()
    qT = [np.ascontiguousarray(q[b].T) for b in range(B)]
    kT = [np.ascontiguousarray(k[b].T) for b in range(B)]
    vT = [np.ascontiguousarray(v[b].T) for b in range(B)]
    in_maps = []
    for c in range(N_CORES):
        b, g = c // 4, c % 4
        cs = CS * g
        in_maps.append({
            "qT": qT[b], "kT": kT[b], "vT": vT[b],
            "wq": np.ascontiguousarray(Wq[:, cs:cs + CS]),
            "wk": np.ascontiguousarray(Wk[:, cs:cs + CS]),
            "wv": np.ascontiguousarray(Wv[:, cs:cs + CS]),
            "wo": np.ascontiguousarray(Wo[cs:cs + CS, :]),
            "bq": np.ascontiguousarray(bq[cs:cs + CS]),
        })
    res = run_bass_kernel_spmd(nc, in_maps, list(range(N_CORES)))

    bo_eff = bo.astype(np.float64) + bv.astype(np.float64) @ Wo.astype(np.float64)
    out = np.empty((B, T, D), np.float32)
    for b in range(B):
        acc = np.zeros((T, D), np.float64)
        for g in range(HPC):
            acc += res.results[4 * b + g]["out"].astype(np.float64)
        out[b] = (acc + bo_eff).astype(np.float32)
    return out
